# revision 19
# baseline (speedup 1.0000x reference)
"""LogoAwareAttention Trainium2 kernel (v2 schedule).

The "logo bias" (geo_bias*geometric + ...) is constant along the softmax
axis, so softmax(x + c) == softmax(x) and the module is plain MHA:

    y = softmax((x Wq)(x Wk)^T / sqrt(Dh)) (x Wv) Wproj + b_proj

Sharding: data-parallel over batch. B=8 -> one batch element per core.

Engine floors per core (N=1024, C=768, H=12, Dh=64, bf16 matmuls):
  PE:  294912 moving-cols @2.4GHz = 122.9us
       (QKV 110592 + S 49152 [two heads paired via PE row groups
        (0,0)/(64,0)] + PV 98304 [M=65: ones|V stationary col emits the
        softmax denominator for free] + proj 36864)
  ACT: 96 exps x [128,1024] ~1.10us = 106us   <- the two binding engines

v2 changes vs v1 (208us measured):
  1. PSUM re-partition (8 banks): S ping-pong 2x[128,1024] (4 banks) +
     PV accumulators 2x[128,512] HALF-QUERY-WIDTH (2 banks) + 2 dedicated
     insert/V banks.  v1's QKV-insert matmuls stole the S->exp ping-pong
     slots, stalling the ACT exp stream ~25-30%.
  2. Half-width PV: each (head, query-half) accumulates in ONE bank with
     stationary [ones|V_h] (den at out partition 0); normalize runs per
     half: den-copy (DVE) -> reciprocal_approx_fast [1,1024 both heads]
     (DVE) -> partition_broadcast (GpSimd) -> mult into outT (DVE).
  3. Consolidated input DMA: 9 big dispatches over 4 queues in first-use
     order (v1: 57 dispatches, 0.65us dispatch serialization each; first
     exp at ~31us -> target ~15us).
  4. Global filler deque: the PE stream is one slot loop per exp-slot
     (S-pair emission) draining ~3us-granular work units (PV halves,
     QKV inserts, V tiles, early projection) in between, so the PE never
     parks while ACT is the floor, and S always lands just-in-time.
  5. Projection: it0 partials run inside phase 5 on the insert banks;
     the rest pipelines 2-deep on the freed S banks after the last exp,
     with outT[5]-dependent cc=5 matmuls deferred.
"""

import numpy as np
import ml_dtypes
from collections import deque

import concourse.bass as bass
import concourse.tile as tile
from concourse import bacc, mybir
from concourse.bass_utils import run_bass_kernel_spmd

BF16 = mybir.dt.bfloat16
F32 = mybir.dt.float32
NP_BF16 = ml_dtypes.bfloat16

N = 1024          # tokens
C = 768           # channels
H = 12            # heads
DH = 64           # head dim
CT = C // 128     # 6 c-chunks
TT = N // 128     # 8 token tiles / key tiles
PAIRS = H // 2    # 6 head pairs


def _build_nc():
    nc = bacc.Bacc("TRN2", target_bir_lowering=False, debug=False)

    xt_d = nc.dram_tensor("xt", [C, N], BF16, kind="ExternalInput")
    wqkv_d = nc.dram_tensor("wqkv", [C, 3 * C], BF16, kind="ExternalInput")
    wproj_d = nc.dram_tensor("wproj", [C, C], BF16, kind="ExternalInput")
    bias_d = nc.dram_tensor("bias", [128, C], F32, kind="ExternalInput")
    y_d = nc.dram_tensor("y", [N, C], F32, kind="ExternalOutput")

    with tile.TileContext(nc) as tc:
        with tc.tile_pool(name="const", bufs=1) as cpool, \
             tc.tile_pool(name="qkv", bufs=1) as qkvpool, \
             tc.tile_pool(name="pt", bufs=32) as ptpool, \
             tc.tile_pool(name="norm", bufs=2) as npool, \
             tc.tile_pool(name="s", bufs=3, space="PSUM") as spool, \
             tc.tile_pool(name="acc", bufs=2, space="PSUM") as accpool:

            # prepay the ACT exp table load
            dummy = cpool.tile([1, 2], F32, tag="dummy")
            nc.vector.memset(dummy[0:1, 0:1], 0.0)
            nc.scalar.activation(dummy[0:1, 1:2], dummy[0:1, 0:1],
                                 mybir.ActivationFunctionType.Exp)

            # ---- input DMA: few big dispatches, first-use order ----------
            # sync:   xt chunks 0-2, then wproj, bias (tail-only)
            # scalar: xt chunks 3-5 (ACT idle until ~15us)
            # gpsimd: W_qkv cols ft0 (q0), ft6 (k0), V cols, q-rest, k-rest
            xt_sb = cpool.tile([128, CT * N], BF16, tag="xt")
            x3 = xt_sb[:].rearrange("p (c n) -> p c n", n=N)
            for cc in range(CT):
                eng = nc.sync if cc % 2 == 0 else nc.scalar
                eng.dma_start(x3[:, cc, :], xt_d[cc * 128:(cc + 1) * 128, :])

            wq_sb = cpool.tile([128, CT * 3 * C], BF16, tag="wqkv")
            w3 = wq_sb[:].rearrange("p (c f) -> p c f", f=3 * C)

            def wdma(eng, c0, c1):
                eng.dma_start(
                    w3[:, :, c0:c1],
                    wqkv_d[:, c0:c1].rearrange("(c p) f -> p c f", p=128))

            wdma(nc.gpsimd, 0, 128)          # q features of pair 0
            wdma(nc.gpsimd, 768, 896)        # k features of pair 0
            wdma(nc.gpsimd, 1536, 2304)      # V columns
            wdma(nc.gpsimd, 128, 768)        # q features pairs 1-5
            wdma(nc.gpsimd, 896, 1536)       # k features pairs 1-5

            wp_sb = cpool.tile([128, CT * C], BF16, tag="wproj")
            wp3 = wp_sb[:].rearrange("p (c f) -> p c f", f=C)
            nc.sync.dma_start(
                wp3[:, :, :], wproj_d[:, :].rearrange("(c p) f -> p c f", p=128))
            bias_sb = cpool.tile([128, C], F32, tag="bias")
            nc.sync.dma_start(bias_sb[:], bias_d[:, :])

            # ---- persistent SBUF tiles -----------------------------------
            qkT = [None] * 12          # ft 0..5 = q pairs, 6..11 = k pairs
            v_sb = [None] * TT         # [128, 12*65] = per head [V | ones]
            pt_tab = {}                # (f, jt) -> (pT_a, pT_b)
            outT = [qkvpool.tile([128, N], BF16, tag=f"outT{i}",
                                 name=f"outT{i}") for i in range(PAIRS)]

            # ---- building blocks -----------------------------------------
            def emit_qk_warm(ft):
                """Full feature tile through a [128,1024] S-bank pair
                (warmup only, before the S ping-pong starts)."""
                ps = spool.tile([128, N], F32, tag="s", name=f"qkw{ft}")
                t = qkvpool.tile([128, N], BF16, tag=f"qk{ft}", name=f"qk{ft}")
                qkT[ft] = t
                for half in range(2):
                    sl = slice(half * 512, (half + 1) * 512)
                    for cc in range(CT):
                        nc.tensor.matmul(
                            ps[:, sl],
                            lhsT=w3[:, cc, ft * 128:(ft + 1) * 128],
                            rhs=x3[:, cc, sl],
                            start=(cc == 0), stop=(cc == CT - 1))
                    nc.vector.tensor_copy(out=t[:, sl], in_=ps[:, sl])

            ins_ps = {}

            def emit_qk_insert_part(ft, half):
                """One 512-token half of a q/k feature tile; both halves
                share one S-rotation slot (allocated at half 0)."""
                if half == 0:
                    ins_ps[ft] = spool.tile([128, N], F32, tag="s",
                                            name=f"qki{ft}")
                    qkT[ft] = qkvpool.tile([128, N], BF16, tag=f"qk{ft}",
                                           name=f"qk{ft}")
                ps = ins_ps[ft]
                sl = slice(half * 512, (half + 1) * 512)
                for cc in range(CT):
                    nc.tensor.matmul(
                        ps[:, sl],
                        lhsT=w3[:, cc, ft * 128:(ft + 1) * 128],
                        rhs=x3[:, cc, sl],
                        start=(cc == 0), stop=(cc == CT - 1))
                nc.vector.tensor_copy(out=qkT[ft][:, sl], in_=ps[:, sl])

            v_ps2 = {}

            def emit_v_part(tt, part):
                """V for token tile tt on one S-rotation slot, split in
                two cc-group units; 512/256-col matmuls cc-interleaved so
                the 256-col LDWEIGHTS hide under the 512-col streams."""
                if part == 0:
                    v_ps2[tt] = spool.tile([128, N], F32, tag="s",
                                           name=f"v{tt}")
                ps = v_ps2[tt]
                for cc in (range(3) if part == 0 else range(3, CT)):
                    nc.tensor.matmul(
                        ps[:, 0:512],
                        lhsT=x3[:, cc, tt * 128:(tt + 1) * 128],
                        rhs=w3[:, cc, 2 * C:2 * C + 512],
                        start=(cc == 0), stop=(cc == CT - 1))
                    nc.tensor.matmul(
                        ps[:, 512:768],
                        lhsT=x3[:, cc, tt * 128:(tt + 1) * 128],
                        rhs=w3[:, cc, 2 * C + 512:3 * C],
                        start=(cc == 0), stop=(cc == CT - 1))
                if part == 1:
                    t = qkvpool.tile([128, H * 65], BF16, tag=f"v{tt}",
                                     name=f"v{tt}")
                    v_sb[tt] = t
                    t3 = t[:].rearrange("p (h w) -> p h w", w=65)
                    nc.vector.memset(t3[:, :, 64:65], 1.0)
                    nc.vector.tensor_copy(
                        out=t3[:, :, 0:64],
                        in_=ps[:, 0:768].rearrange("p (h w) -> p h w", w=64))

            def emit_s_pair(f, jt):
                """S^T for heads (2f, 2f+1) key-tile jt: two K=64 matmuls
                run concurrently on PE row groups (0,0)/(64,0); exp each."""
                qt, kt = qkT[f], qkT[6 + f]
                a, b = 2 * f, 2 * f + 1
                ps_a = spool.tile([128, N], F32, tag="s", name=f"sa{f}_{jt}")
                ps_b = spool.tile([128, N], F32, tag="s", name=f"sb{f}_{jt}")
                jcols = slice(jt * 128, (jt + 1) * 128)
                for half in range(2):
                    sl = slice(half * 512, (half + 1) * 512)
                    nc.tensor.matmul(
                        ps_a[:, sl], lhsT=kt[0:64, jcols], rhs=qt[0:64, sl],
                        start=True, stop=True, tile_position=(0, 0))
                for half in range(2):
                    sl = slice(half * 512, (half + 1) * 512)
                    nc.tensor.matmul(
                        ps_b[:, sl], lhsT=kt[64:128, jcols], rhs=qt[64:128, sl],
                        start=True, stop=True, tile_position=(64, 0))
                pa = ptpool.tile([128, N], BF16, tag="pT", name=f"pT{a}_{jt}")
                nc.scalar.activation(pa[:], ps_a[:],
                                     mybir.ActivationFunctionType.Exp)
                pb = ptpool.tile([128, N], BF16, tag="pT", name=f"pT{b}_{jt}")
                nc.scalar.activation(pb[:], ps_b[:],
                                     mybir.ActivationFunctionType.Exp)
                pt_tab[(f, jt)] = (pa, pb)

            def emit_pv(p, jt, half, accs):
                """PV for both heads of pair p, key-tile jt, query half.
                acc[0:64] = out, acc[64] = denominator (ones col)."""
                hs = slice(half * 512, (half + 1) * 512)
                for hh in range(2):
                    h = 2 * p + hh
                    nc.tensor.matmul(
                        accs[hh][0:65, :],
                        lhsT=v_sb[jt][:, h * 65:(h + 1) * 65],
                        rhs=pt_tab[(p, jt)][hh][:, hs],
                        start=(jt == 0), stop=(jt == TT - 1))

            def norm_half(p, half, accs):
                """den-copy -> recip (both heads batched) -> partition
                broadcast -> multiply into outT[p]."""
                den = npool.tile([1, N], F32, tag="den", name=f"den{p}_{half}")
                nc.vector.tensor_copy(out=den[0:1, 0:512], in_=accs[0][64:65, :])
                nc.vector.tensor_copy(out=den[0:1, 512:N], in_=accs[1][64:65, :])
                recip = npool.tile([1, N], F32, tag="recip",
                                   name=f"recip{p}_{half}")
                nc.vector.reciprocal_approx_fast(out=recip[:], in_=den[:])
                bc = npool.tile([64, N], F32, tag="bc", name=f"bc{p}_{half}")
                nc.gpsimd.partition_broadcast(bc[0:64, 0:512], recip[0:1, 0:512])
                nc.gpsimd.partition_broadcast(bc[0:64, 512:N], recip[0:1, 512:N])
                hc = slice(half * 512, (half + 1) * 512)
                nc.vector.tensor_tensor(
                    out=outT[p][0:64, hc], in0=accs[0][0:64, :],
                    in1=bc[0:64, 0:512], op=mybir.AluOpType.mult)
                nc.vector.tensor_tensor(
                    out=outT[p][64:128, hc], in0=accs[1][0:64, :],
                    in1=bc[0:64, 512:N], op=mybir.AluOpType.mult)

            # ---- filler deque: PE work drained between S emissions -------
            fill = deque()   # items: (cost_cycles, closure, ready_pred|None)

            def drain(budget):
                while fill and budget > 0:
                    c, fn, ready = fill[0]
                    if c > budget + 1024:
                        break
                    if ready is not None and not ready():
                        break
                    fill.popleft()
                    fn()
                    budget -= c

            def push_pv_half(p, half):
                """PV + normalize units for pair p, one query half."""
                accs = [None, None]

                def mk_pv(jt):
                    def fn():
                        if jt == 0:
                            accs[0] = accpool.tile(
                                [128, 512], F32, tag="acc",
                                name=f"acc{2 * p}_{half}")
                            accs[1] = accpool.tile(
                                [128, 512], F32, tag="acc",
                                name=f"acc{2 * p + 1}_{half}")
                        emit_pv(p, jt, half, accs)
                    return fn
                for jt in range(TT):
                    fill.append((1024, mk_pv(jt),
                                 (lambda jt=jt: (p, jt) in pt_tab)))
                fill.append((0, lambda: norm_half(p, half, accs), None))

            def push_pv_phase(p):
                push_pv_half(p, 0)
                push_pv_half(p, 1)

            def push_inserts(f1):
                for ft in (f1, 6 + f1):
                    for half in range(2):
                        fill.append((3072, lambda ft=ft, half=half:
                                     emit_qk_insert_part(ft, half), None))

            def push_v(tt):
                fill.append((2304, lambda: emit_v_part(tt, 0), None))
                fill.append((2304, lambda: emit_v_part(tt, 1), None))

            # ---- projection helpers --------------------------------------
            proj_ps = {}

            def y_out(it, pieces):
                """bias add (DVE/GpSimd alternating) + store, DMA spread
                over three queues so the tail transfers overlap."""
                y_sb = npool.tile([128, C], F32, tag="ysb", bufs=3,
                                  name=f"y{it}")
                eng = nc.vector
                if len(pieces) == 1:
                    eng.tensor_tensor(out=y_sb[:], in0=pieces[0],
                                      in1=bias_sb[:], op=mybir.AluOpType.add)
                else:
                    eng.tensor_tensor(out=y_sb[:, 0:512], in0=pieces[0],
                                      in1=bias_sb[:, 0:512],
                                      op=mybir.AluOpType.add)
                    eng.tensor_tensor(out=y_sb[:, 512:C], in0=pieces[1],
                                      in1=bias_sb[:, 512:C],
                                      op=mybir.AluOpType.add)
                dq = (nc.sync, nc.scalar, nc.gpsimd)[it % 3]
                dq.dma_start(y_d[it * 128:(it + 1) * 128, :], y_sb[:])

            def proj_start(it, ccs, pool, tag):
                ps = pool.tile([128, N if tag == "s" else 512], F32, tag=tag,
                               name=f"proj{it}")
                proj_ps[it] = ps
                for cc in ccs:
                    for sl in (slice(0, 512), slice(512, 768)):
                        nc.tensor.matmul(
                            ps[:, sl],
                            lhsT=outT[cc][:, it * 128:(it + 1) * 128],
                            rhs=wp3[:, cc, sl],
                            start=(cc == 0), stop=False)

            def proj_finish(it):
                ps = proj_ps[it]
                for sl in (slice(0, 512), slice(512, 768)):
                    nc.tensor.matmul(
                        ps[:, sl],
                        lhsT=outT[CT - 1][:, it * 128:(it + 1) * 128],
                        rhs=wp3[:, CT - 1, sl],
                        start=False, stop=True)
                y_out(it, (ps[:, 0:C],))

            # projection on a pair of single banks ([128,512] pool tiles,
            # 512 + 256 columns) -- used for it0 inside phase 5 on the
            # insert banks, and for the tail rotation on acc/insert banks.
            def proj_start_pair(it, pool, tag):
                ps1 = pool.tile([128, 512], F32, tag=tag, name=f"proj{it}a")
                ps2 = pool.tile([128, 512], F32, tag=tag, name=f"proj{it}b")
                proj_ps[it] = (ps1, ps2)
                tcols = slice(it * 128, (it + 1) * 128)
                for cc in range(CT - 1):
                    for ps, sl in ((ps1, slice(0, 512)), (ps2, slice(512, 768))):
                        w = sl.stop - sl.start
                        nc.tensor.matmul(
                            ps[:, 0:w],
                            lhsT=outT[cc][:, tcols],
                            rhs=wp3[:, cc, sl],
                            start=(cc == 0), stop=False)

            def proj_finish_pair(it):
                ps1, ps2 = proj_ps[it]
                tcols = slice(it * 128, (it + 1) * 128)
                for ps, sl in ((ps1, slice(0, 512)), (ps2, slice(512, 768))):
                    w = sl.stop - sl.start
                    nc.tensor.matmul(
                        ps[:, 0:w],
                        lhsT=outT[CT - 1][:, tcols],
                        rhs=wp3[:, CT - 1, sl],
                        start=False, stop=True)
                y_out(it, (ps1[:, 0:512], ps2[:, 0:256]))

            # ---- schedule ------------------------------------------------
            # PE clock priming during the input DMA window
            prime = cpool.tile([128, 512], BF16, tag="prime")
            nc.vector.memset(prime[:, :], 0.0)
            pps = spool.tile([128, N], F32, tag="s", name="prime_ps")
            for i in range(14):
                nc.tensor.matmul(pps[:, 0:512], lhsT=prime[:, 0:128],
                                 rhs=prime[:, :], start=(i == 0),
                                 stop=(i == 13))

            # warmup: q/k features of pair 0 through the S banks
            emit_qk_warm(0)
            emit_qk_warm(6)

            # exp-slot cadence: ACT does 2x1104ns per slot = ~5300 PE cycles;
            # S-pair itself is 2048 -> ~3300 filler cycles per slot.
            SLOT_FILLER = 3600

            for f in range(PAIRS):
                if f == 0:
                    # pair-1 inserts early (they gate S(1,0)); V fills the rest
                    push_v(0)
                    push_inserts(1)
                    for tt in range(1, 5):
                        push_v(tt)
                elif f == 1:
                    for tt in range(5, TT):
                        push_v(tt)
                if 1 <= f < PAIRS - 1:
                    push_inserts(f + 1)
                if f >= 1:
                    push_pv_phase(f - 1)

                for jt in range(TT):
                    if f == 0 and jt <= 1:
                        # pipeline not primed: S first, tiny filler after
                        emit_s_pair(f, jt)
                        drain(SLOT_FILLER - 2048)
                    else:
                        drain(SLOT_FILLER)
                        emit_s_pair(f, jt)
                    if f == PAIRS - 1 and jt == 1:
                        # pair 5's first PV half slides into phase 5; the
                        # ready-predicate keeps units behind their exps
                        push_pv_half(PAIRS - 1, 0)

            # last pair's remaining PV + the projection tail: 4 PSUM
            # streams (2 S-bank pairs + acc pair + insert pair) keep the
            # PE dense; cc=5 (outT[5]) matmuls deferred per stream.
            push_pv_half(PAIRS - 1, 1)
            drain(1 << 30)
            proj_start(0, range(CT - 1), spool, "s")
            proj_start(1, range(CT - 1), spool, "s")
            proj_start(2, range(CT - 1), spool, "s")
            proj_start_pair(3, accpool, "acc")
            proj_finish(0)
            proj_start(4, range(CT - 1), spool, "s")
            proj_finish(1)
            proj_start(5, range(CT - 1), spool, "s")
            proj_finish(2)
            proj_start(6, range(CT - 1), spool, "s")
            proj_finish_pair(3)
            proj_start_pair(7, accpool, "acc")
            proj_finish(4)
            proj_finish(5)
            proj_finish(6)
            proj_finish_pair(7)

    nc.compile()
    return nc


_NC_CACHE = None


def _get_nc():
    global _NC_CACHE
    if _NC_CACHE is None:
        _NC_CACHE = _build_nc()
    return _NC_CACHE


def kernel(x, geometric, text, color, W_qkv, W_proj, b_proj,
           geo_bias, txt_bias, col_bias, _trace=False, **_ignored):
    x = np.asarray(x, dtype=np.float32)
    W_qkv = np.asarray(W_qkv, dtype=np.float32)
    W_proj = np.asarray(W_proj, dtype=np.float32)
    b_proj = np.asarray(b_proj, dtype=np.float32)

    scale = DH ** -0.5
    wqkv = W_qkv.copy()
    wqkv[:, :C] *= scale
    wqkv_bf = wqkv.astype(NP_BF16)
    wproj_bf = W_proj.astype(NP_BF16)
    bias_f = np.ascontiguousarray(
        np.broadcast_to(b_proj, (128, C))).astype(np.float32)

    in_maps = []
    for b in range(8):
        xt = np.ascontiguousarray(x[b].T).astype(NP_BF16)
        in_maps.append({"xt": xt, "wqkv": wqkv_bf, "wproj": wproj_bf,
                        "bias": bias_f})

    nc = _get_nc()
    res = run_bass_kernel_spmd(nc, in_maps, core_ids=list(range(8)),
                               trace=_trace)
    y = np.stack([r["y"] for r in res.results]).astype(np.float32)
    if _trace:
        kernel.last_results = res
    return y


# revision 22
# speedup vs baseline: 1.1153x; 1.1153x over previous
"""LogoAwareAttention Trainium2 kernel (v7 schedule).

The "logo bias" (geo_bias*geometric + ...) is constant along the softmax
axis, so softmax(x + c) == softmax(x) and the module is plain MHA:

    y = softmax((x Wq)(x Wk)^T / sqrt(Dh)) (x Wv) Wproj + b_proj

Sharding: data-parallel over batch. B=8 -> one batch element per core.

Engine floors per core (N=1024, C=768, H=12, Dh=64, bf16 matmuls):
  PE:  294912 moving-cols @2.4GHz = 122.9us
       (QKV 110592 + S 49152 [two heads paired via PE row groups
        (0,0)/(64,0)] + PV 98304 [M=65: V|ones stationary col emits the
        softmax denominator for free] + proj 36864)
  ACT: 96 exps x [128,1024] ~1.10us = 106us

Design notes (what the traces taught us):
  * PSUM (8 banks): S ping-pong 2x[128,1024] (4) + two persistent
    [128,512] PV accumulators (2) + 2 insert/V banks.  Dedicated insert
    banks keep QKV work off the S->exp ping-pong (v1's stall source).
  * Half-query-width PV accumulation: acc[0:64]=out^T, acc[64]=den
    (stationary [V_h|ones]); normalize per (pair, half): den-copy (DVE)
    -> reciprocal_approx_fast [1,1024] (DVE) -> partition_broadcast
    (GpSimd) -> 2 mults into outT (DVE).
  * HAM keep-alive: the PE's HAM down-throttles to 1.2 GHz after
    micro-idles (observed 3.4us half-clock bursts after every S-wait).
    When the filler deque runs short of a slot's budget, we emit dummy
    512-col matmuls into dead partitions (96) of the persistent acc
    banks to keep the activity window hot.  PE idle is converted into
    clock-keeping, so real matmuls stay at 2.4 GHz.
  * 256-col matmuls pay their ~107ns LDWEIGHTS unhidden; V interleaves
    its 512/256-col streams per cc so the LDW hides under the 512s.
  * Consolidated input DMA (11 dispatches over 3 queues, first-use
    order); PE clock priming matmuls run during the DMA window.
  * Tail: it0 partials inside phase 5 on the insert banks; remaining
    its pipeline over S banks + acc banks + insert banks (4 streams),
    with outT[5]-dependent cc=5 matmuls deferred; y bias-adds on DVE,
    output DMA spread over three queues.
"""

import numpy as np
import ml_dtypes
from collections import deque

import concourse.bass as bass
import concourse.tile as tile
from concourse import bacc, mybir
from concourse.bass_utils import run_bass_kernel_spmd

BF16 = mybir.dt.bfloat16
F32 = mybir.dt.float32
NP_BF16 = ml_dtypes.bfloat16

N = 1024          # tokens
C = 768           # channels
H = 12            # heads
DH = 64           # head dim
CT = C // 128     # 6 c-chunks
TT = N // 128     # 8 token tiles / key tiles
PAIRS = H // 2    # 6 head pairs


def _build_nc():
    nc = bacc.Bacc("TRN2", target_bir_lowering=False, debug=False)

    xt_d = nc.dram_tensor("xt", [C, N], BF16, kind="ExternalInput")
    wqkv_d = nc.dram_tensor("wqkv", [C, 3 * C], BF16, kind="ExternalInput")
    wproj_d = nc.dram_tensor("wproj", [C, C], BF16, kind="ExternalInput")
    bias_d = nc.dram_tensor("bias", [128, C], F32, kind="ExternalInput")
    y_d = nc.dram_tensor("y", [N, C], F32, kind="ExternalOutput")

    with tile.TileContext(nc) as tc:
        with tc.tile_pool(name="const", bufs=1) as cpool, \
             tc.tile_pool(name="qkv", bufs=1) as qkvpool, \
             tc.tile_pool(name="pt", bufs=32) as ptpool, \
             tc.tile_pool(name="norm", bufs=2) as npool, \
             tc.tile_pool(name="s", bufs=2, space="PSUM") as spool, \
             tc.tile_pool(name="acc", bufs=1, space="PSUM") as accpool, \
             tc.tile_pool(name="ins", bufs=2, space="PSUM") as ipool:

            # prepay the ACT exp table load
            dummy = cpool.tile([1, 2], F32, tag="dummy")
            nc.vector.memset(dummy[0:1, 0:1], 0.0)
            nc.scalar.activation(dummy[0:1, 1:2], dummy[0:1, 0:1],
                                 mybir.ActivationFunctionType.Exp)

            # ---- input DMA: few big dispatches, first-use order ----------
            # sync:   xt chunks 0,2,4 then wproj, bias (tail-only)
            # scalar: xt chunks 1,3,5 (ACT idle until ~15us)
            # gpsimd: W_qkv cols ft0 (q0), ft6 (k0), V cols, q-rest, k-rest
            xt_sb = cpool.tile([128, CT * N], BF16, tag="xt")
            x3 = xt_sb[:].rearrange("p (c n) -> p c n", n=N)
            for cc in range(CT):
                eng = nc.sync if cc % 2 == 0 else nc.scalar
                eng.dma_start(x3[:, cc, :], xt_d[cc * 128:(cc + 1) * 128, :])

            wq_sb = cpool.tile([128, CT * 3 * C], BF16, tag="wqkv")
            w3 = wq_sb[:].rearrange("p (c f) -> p c f", f=3 * C)

            def wdma(eng, c0, c1):
                eng.dma_start(
                    w3[:, :, c0:c1],
                    wqkv_d[:, c0:c1].rearrange("(c p) f -> p c f", p=128))

            wdma(nc.gpsimd, 0, 128)          # q features of pair 0
            wdma(nc.gpsimd, 768, 896)        # k features of pair 0
            wdma(nc.gpsimd, 1536, 2304)      # V columns
            wdma(nc.gpsimd, 128, 768)        # q features pairs 1-5
            wdma(nc.gpsimd, 896, 1536)       # k features pairs 1-5

            wp_sb = cpool.tile([128, CT * C], BF16, tag="wproj")
            wp3 = wp_sb[:].rearrange("p (c f) -> p c f", f=C)
            nc.sync.dma_start(
                wp3[:, :, :], wproj_d[:, :].rearrange("(c p) f -> p c f", p=128))
            bias_sb = cpool.tile([128, C], F32, tag="bias")
            nc.sync.dma_start(bias_sb[:], bias_d[:, :])

            # ---- persistent tiles ----------------------------------------
            qkT = [None] * 12          # ft 0..5 = q pairs, 6..11 = k pairs
            v_sb = [None] * TT         # [128, 12*65] = per head [V | ones]
            pt_tab = {}                # (f, jt) -> (pT_a, pT_b)
            outT = [qkvpool.tile([128, N], BF16, tag=f"outT{i}",
                                 name=f"outT{i}") for i in range(PAIRS)]
            # persistent PV accumulators: PV uses partitions 0:65, the
            # HAM keep-alive dummies write partition 96, the tail
            # projection reuses the full 128 partitions.
            accA = accpool.tile([128, 512], F32, tag="accA", name="accA")
            accB = accpool.tile([128, 512], F32, tag="accB", name="accB")

            # PE clock priming source (and dummy matmul operand)
            prime = cpool.tile([128, 512], BF16, tag="prime")
            nc.vector.memset(prime[:, :], 0.0)

            def keepalive(k):
                """Dummy matmuls into dead acc partitions: ~512 cycles of
                clock-keeping each, never read."""
                for _ in range(k):
                    nc.tensor.matmul(
                        accA[96:97, :], lhsT=prime[:, 0:1], rhs=prime[:, :],
                        start=False, stop=True, skip_group_check=True,
                        tile_position=(0, 96))

            # ---- building blocks -----------------------------------------
            def emit_qk_warm(ft):
                ps = spool.tile([128, N], F32, tag="s", name=f"qkw{ft}")
                t = qkvpool.tile([128, N], BF16, tag=f"qk{ft}", name=f"qk{ft}")
                qkT[ft] = t
                for half in range(2):
                    sl = slice(half * 512, (half + 1) * 512)
                    for cc in range(CT):
                        nc.tensor.matmul(
                            ps[:, sl],
                            lhsT=w3[:, cc, ft * 128:(ft + 1) * 128],
                            rhs=x3[:, cc, sl],
                            start=(cc == 0), stop=(cc == CT - 1))
                    nc.vector.tensor_copy(out=t[:, sl], in_=ps[:, sl])

            def emit_qk_insert_half(ft, half):
                ps = ipool.tile([128, 512], F32, tag="i", name=f"qk{ft}h{half}")
                sl = slice(half * 512, (half + 1) * 512)
                for cc in range(CT):
                    nc.tensor.matmul(
                        ps[:, :],
                        lhsT=w3[:, cc, ft * 128:(ft + 1) * 128],
                        rhs=x3[:, cc, sl],
                        start=(cc == 0), stop=(cc == CT - 1))
                if half == 0:
                    qkT[ft] = qkvpool.tile([128, N], BF16, tag=f"qk{ft}",
                                           name=f"qk{ft}")
                nc.vector.tensor_copy(out=qkT[ft][:, sl], in_=ps[:, :])

            v_ps = {}

            def emit_v_part(tt, part):
                """V for token tile tt on the insert-bank pair, as two
                cc-group units; the 512/256-col matmuls are interleaved
                per cc so the 256-col LDWEIGHTS hide."""
                if part == 0:
                    ps1 = ipool.tile([128, 512], F32, tag="i", name=f"v{tt}a")
                    ps2 = ipool.tile([128, 512], F32, tag="i", name=f"v{tt}b")
                    v_ps[tt] = (ps1, ps2)
                ps1, ps2 = v_ps[tt]
                for cc in (range(3) if part == 0 else range(3, CT)):
                    nc.tensor.matmul(
                        ps1[:, 0:512],
                        lhsT=x3[:, cc, tt * 128:(tt + 1) * 128],
                        rhs=w3[:, cc, 2 * C:2 * C + 512],
                        start=(cc == 0), stop=(cc == CT - 1))
                    nc.tensor.matmul(
                        ps2[:, 0:256],
                        lhsT=x3[:, cc, tt * 128:(tt + 1) * 128],
                        rhs=w3[:, cc, 2 * C + 512:3 * C],
                        start=(cc == 0), stop=(cc == CT - 1))
                if part == 1:
                    t = qkvpool.tile([128, H * 65], BF16, tag=f"v{tt}",
                                     name=f"v{tt}")
                    v_sb[tt] = t
                    t3 = t[:].rearrange("p (h w) -> p h w", w=65)
                    nc.vector.memset(t3[:, :, 64:65], 1.0)
                    nc.vector.tensor_copy(
                        out=t3[:, 0:8, 0:64],
                        in_=ps1[:, 0:512].rearrange("p (h w) -> p h w", w=64))
                    nc.vector.tensor_copy(
                        out=t3[:, 8:12, 0:64],
                        in_=ps2[:, 0:256].rearrange("p (h w) -> p h w", w=64))

            def emit_s_pair(f, jt):
                qt, kt = qkT[f], qkT[6 + f]
                a, b = 2 * f, 2 * f + 1
                ps_a = spool.tile([128, N], F32, tag="s", name=f"sa{f}_{jt}")
                ps_b = spool.tile([128, N], F32, tag="s", name=f"sb{f}_{jt}")
                jcols = slice(jt * 128, (jt + 1) * 128)
                for half in range(2):
                    sl = slice(half * 512, (half + 1) * 512)
                    nc.tensor.matmul(
                        ps_a[:, sl], lhsT=kt[0:64, jcols], rhs=qt[0:64, sl],
                        start=True, stop=True, tile_position=(0, 0))
                for half in range(2):
                    sl = slice(half * 512, (half + 1) * 512)
                    nc.tensor.matmul(
                        ps_b[:, sl], lhsT=kt[64:128, jcols], rhs=qt[64:128, sl],
                        start=True, stop=True, tile_position=(64, 0))
                pa = ptpool.tile([128, N], BF16, tag="pT", name=f"pT{a}_{jt}")
                nc.scalar.activation(pa[:], ps_a[:],
                                     mybir.ActivationFunctionType.Exp)
                pb = ptpool.tile([128, N], BF16, tag="pT", name=f"pT{b}_{jt}")
                nc.scalar.activation(pb[:], ps_b[:],
                                     mybir.ActivationFunctionType.Exp)
                pt_tab[(f, jt)] = (pa, pb)

            def emit_pv(p, jt, half):
                """PV for both heads of pair p, key-tile jt, query half.
                acc[0:64] = out^T, acc[64] = denominator (ones col)."""
                hs = slice(half * 512, (half + 1) * 512)
                for hh, acc in ((0, accA), (1, accB)):
                    h = 2 * p + hh
                    nc.tensor.matmul(
                        acc[0:65, :],
                        lhsT=v_sb[jt][:, h * 65:(h + 1) * 65],
                        rhs=pt_tab[(p, jt)][hh][:, hs],
                        start=(jt == 0), stop=(jt == TT - 1))

            def norm_half(p, half):
                den = npool.tile([1, N], F32, tag="den", name=f"den{p}_{half}")
                nc.vector.tensor_copy(out=den[0:1, 0:512], in_=accA[64:65, :])
                nc.vector.tensor_copy(out=den[0:1, 512:N], in_=accB[64:65, :])
                recip = npool.tile([1, N], F32, tag="recip",
                                   name=f"recip{p}_{half}")
                nc.vector.reciprocal_approx_fast(out=recip[:], in_=den[:])
                bc = npool.tile([64, N], F32, tag="bc", name=f"bc{p}_{half}")
                nc.gpsimd.partition_broadcast(bc[0:64, 0:512], recip[0:1, 0:512])
                nc.gpsimd.partition_broadcast(bc[0:64, 512:N], recip[0:1, 512:N])
                hc = slice(half * 512, (half + 1) * 512)
                nc.vector.tensor_tensor(
                    out=outT[p][0:64, hc], in0=accA[0:64, :],
                    in1=bc[0:64, 0:512], op=mybir.AluOpType.mult)
                nc.vector.tensor_tensor(
                    out=outT[p][64:128, hc], in0=accB[0:64, :],
                    in1=bc[0:64, 512:N], op=mybir.AluOpType.mult)

            # ---- filler deque --------------------------------------------
            fill = deque()   # items: (cost_cycles, closure, ready_pred|None)

            def drain(budget):
                while fill and budget > 0:
                    c, fn, ready = fill[0]
                    if c > budget + 2048:
                        break
                    if ready is not None and not ready():
                        break
                    fill.popleft()
                    fn()
                    budget -= c
                return budget

            def ensure_qk(f):
                """Force-drain until pair f's q/k feature tiles exist
                (their insert units are always ahead in the deque)."""
                while qkT[f] is None or qkT[6 + f] is None:
                    c, fn, ready = fill.popleft()
                    fn()

            def push_pv_half(p, half):
                for jt in range(TT):
                    fill.append((1024, lambda jt=jt: emit_pv(p, jt, half),
                                 (lambda jt=jt: (p, jt) in pt_tab)))
                fill.append((0, lambda: norm_half(p, half), None))

            def push_pv_phase(p):
                push_pv_half(p, 0)
                push_pv_half(p, 1)

            def push_inserts(f1):
                for ft in (f1, 6 + f1):
                    for half in range(2):
                        fill.append((3072, lambda ft=ft, half=half:
                                     emit_qk_insert_half(ft, half), None))

            def push_v(tt):
                fill.append((2304, lambda: emit_v_part(tt, 0), None))
                fill.append((2304, lambda: emit_v_part(tt, 1), None))

            # ---- projection helpers --------------------------------------
            proj_ps = {}

            def y_out(it, pieces):
                y_sb = npool.tile([128, C], F32, tag="ysb", bufs=3,
                                  name=f"y{it}")
                if len(pieces) == 1:
                    nc.vector.tensor_tensor(out=y_sb[:], in0=pieces[0],
                                            in1=bias_sb[:],
                                            op=mybir.AluOpType.add)
                else:
                    nc.vector.tensor_tensor(out=y_sb[:, 0:512], in0=pieces[0],
                                            in1=bias_sb[:, 0:512],
                                            op=mybir.AluOpType.add)
                    nc.vector.tensor_tensor(out=y_sb[:, 512:C], in0=pieces[1],
                                            in1=bias_sb[:, 512:C],
                                            op=mybir.AluOpType.add)
                dq = (nc.sync, nc.scalar, nc.gpsimd)[it % 3]
                dq.dma_start(y_d[it * 128:(it + 1) * 128, :], y_sb[:])

            def proj_start(it, ccs):
                ps = spool.tile([128, N], F32, tag="s", name=f"proj{it}")
                proj_ps[it] = ps
                for cc in ccs:
                    for sl in (slice(0, 512), slice(512, 768)):
                        nc.tensor.matmul(
                            ps[:, sl],
                            lhsT=outT[cc][:, it * 128:(it + 1) * 128],
                            rhs=wp3[:, cc, sl],
                            start=(cc == 0), stop=False)

            def proj_finish(it):
                ps = proj_ps[it]
                for sl in (slice(0, 512), slice(512, 768)):
                    nc.tensor.matmul(
                        ps[:, sl],
                        lhsT=outT[CT - 1][:, it * 128:(it + 1) * 128],
                        rhs=wp3[:, CT - 1, sl],
                        start=False, stop=True)
                y_out(it, (ps[:, 0:C],))

            def proj_start_pair(it, ps1, ps2):
                proj_ps[it] = (ps1, ps2)
                tcols = slice(it * 128, (it + 1) * 128)
                for cc in range(CT - 1):
                    for ps, sl in ((ps1, slice(0, 512)), (ps2, slice(512, 768))):
                        w = sl.stop - sl.start
                        nc.tensor.matmul(
                            ps[:, 0:w],
                            lhsT=outT[cc][:, tcols],
                            rhs=wp3[:, cc, sl],
                            start=(cc == 0), stop=False)

            def proj_finish_pair(it):
                ps1, ps2 = proj_ps[it]
                tcols = slice(it * 128, (it + 1) * 128)
                for ps, sl in ((ps1, slice(0, 512)), (ps2, slice(512, 768))):
                    w = sl.stop - sl.start
                    nc.tensor.matmul(
                        ps[:, 0:w],
                        lhsT=outT[CT - 1][:, tcols],
                        rhs=wp3[:, CT - 1, sl],
                        start=False, stop=True)
                y_out(it, (ps1[:, 0:512], ps2[:, 0:256]))

            def proj0_start():
                ps1 = ipool.tile([128, 512], F32, tag="i", name="proj0a")
                ps2 = ipool.tile([128, 512], F32, tag="i", name="proj0b")
                proj_start_pair(0, ps1, ps2)

            # ---- schedule ------------------------------------------------
            # PE clock priming during the input DMA window
            pps = ipool.tile([128, 512], F32, tag="i", name="prime_ps")
            for i in range(14):
                nc.tensor.matmul(pps[:, :], lhsT=prime[:, 0:128],
                                 rhs=prime[:, :], start=(i == 0),
                                 stop=(i == 13))

            emit_qk_warm(0)
            emit_qk_warm(6)

            # exp-slot cadence: ACT does 2x1104ns per slot = ~5300 PE
            # cycles; S-pair itself is 2048 -> ~3300 filler cycles.  The
            # shortfall (deque dry or blocked) becomes HAM keep-alive
            # dummies so the PE activity window never cools.
            SLOT_FILLER = 3300

            for f in range(PAIRS):
                if f == 0:
                    push_v(0)
                    push_inserts(1)
                    for tt in range(1, 5):
                        push_v(tt)
                elif f == 1:
                    for tt in range(5, TT):
                        push_v(tt)
                if 1 <= f < PAIRS - 1:
                    push_inserts(f + 1)
                if f >= 1:
                    push_pv_phase(f - 1)
                if f == PAIRS - 1:
                    fill.append((3840, proj0_start, None))
                ensure_qk(f)
                for jt in range(TT):
                    if f == 0 and jt <= 1:
                        emit_s_pair(f, jt)
                        drain(SLOT_FILLER - 2048)
                    else:
                        left = drain(SLOT_FILLER)
                        if f >= 1 and left > 512:
                            keepalive(min(left // 512, 4))
                        emit_s_pair(f, jt)
                    if f == PAIRS - 1 and jt == 1:
                        push_pv_half(PAIRS - 1, 0)

            # last pair's remaining PV + the projection tail
            push_pv_half(PAIRS - 1, 1)
            drain(1 << 30)
            proj_start(1, range(CT - 1))
            proj_start(2, range(CT - 1))
            proj_finish_pair(0)
            proj_start_pair(3, accA, accB)
            proj_finish(1)
            proj_start(5, range(CT - 1))
            proj_finish(2)
            proj_start(6, range(CT - 1))
            proj_finish_pair(3)
            ps4a = ipool.tile([128, 512], F32, tag="i", name="proj4a")
            ps4b = ipool.tile([128, 512], F32, tag="i", name="proj4b")
            proj_start_pair(4, ps4a, ps4b)
            proj_finish(5)
            proj_start(7, range(CT - 1))
            proj_finish_pair(4)
            proj_finish(6)
            proj_finish(7)

    nc.compile()
    return nc


_NC_CACHE = None


def _get_nc():
    global _NC_CACHE
    if _NC_CACHE is None:
        _NC_CACHE = _build_nc()
    return _NC_CACHE


def kernel(x, geometric, text, color, W_qkv, W_proj, b_proj,
           geo_bias, txt_bias, col_bias, _trace=False, **_ignored):
    x = np.asarray(x, dtype=np.float32)
    W_qkv = np.asarray(W_qkv, dtype=np.float32)
    W_proj = np.asarray(W_proj, dtype=np.float32)
    b_proj = np.asarray(b_proj, dtype=np.float32)

    scale = DH ** -0.5
    wqkv = W_qkv.copy()
    wqkv[:, :C] *= scale
    wqkv_bf = wqkv.astype(NP_BF16)
    wproj_bf = W_proj.astype(NP_BF16)
    bias_f = np.ascontiguousarray(
        np.broadcast_to(b_proj, (128, C))).astype(np.float32)

    in_maps = []
    for b in range(8):
        xt = np.ascontiguousarray(x[b].T).astype(NP_BF16)
        in_maps.append({"xt": xt, "wqkv": wqkv_bf, "wproj": wproj_bf,
                        "bias": bias_f})

    nc = _get_nc()
    res = run_bass_kernel_spmd(nc, in_maps, core_ids=list(range(8)),
                               trace=_trace)
    y = np.stack([r["y"] for r in res.results]).astype(np.float32)
    if _trace:
        kernel.last_results = res
    return y


# revision 23
# speedup vs baseline: 1.1895x; 1.0665x over previous
"""LogoAwareAttention Trainium2 kernel (v7 schedule).

The "logo bias" (geo_bias*geometric + ...) is constant along the softmax
axis, so softmax(x + c) == softmax(x) and the module is plain MHA:

    y = softmax((x Wq)(x Wk)^T / sqrt(Dh)) (x Wv) Wproj + b_proj

Sharding: data-parallel over batch. B=8 -> one batch element per core.

Engine floors per core (N=1024, C=768, H=12, Dh=64, bf16 matmuls):
  PE:  294912 moving-cols @2.4GHz = 122.9us
       (QKV 110592 + S 49152 [two heads paired via PE row groups
        (0,0)/(64,0)] + PV 98304 [M=65: V|ones stationary col emits the
        softmax denominator for free] + proj 36864)
  ACT: 96 exps x [128,1024] ~1.10us = 106us

Design notes (what the traces taught us):
  * PSUM (8 banks): S ping-pong 2x[128,1024] (4) + two persistent
    [128,512] PV accumulators (2) + 2 insert/V banks.  Dedicated insert
    banks keep QKV work off the S->exp ping-pong (v1's stall source).
  * Half-query-width PV accumulation: acc[0:64]=out^T, acc[64]=den
    (stationary [V_h|ones]); normalize per (pair, half): den-copy (DVE)
    -> reciprocal_approx_fast [1,1024] (DVE) -> partition_broadcast
    (GpSimd) -> 2 mults into outT (DVE).
  * HAM keep-alive: the PE's HAM down-throttles to 1.2 GHz after
    micro-idles (observed 3.4us half-clock bursts after every S-wait).
    When the filler deque runs short of a slot's budget, we emit dummy
    512-col matmuls into dead partitions (96) of the persistent acc
    banks to keep the activity window hot.  PE idle is converted into
    clock-keeping, so real matmuls stay at 2.4 GHz.
  * 256-col matmuls pay their ~107ns LDWEIGHTS unhidden; V interleaves
    its 512/256-col streams per cc so the LDW hides under the 512s.
  * Consolidated input DMA (11 dispatches over 3 queues, first-use
    order); PE clock priming matmuls run during the DMA window.
  * Tail: it0 partials inside phase 5 on the insert banks; remaining
    its pipeline over S banks + acc banks + insert banks (4 streams),
    with outT[5]-dependent cc=5 matmuls deferred; y bias-adds on DVE,
    output DMA spread over three queues.
"""

import numpy as np
import ml_dtypes
from collections import deque

import concourse.bass as bass
import concourse.tile as tile
from concourse import bacc, mybir
from concourse.bass_utils import run_bass_kernel_spmd

BF16 = mybir.dt.bfloat16
F32 = mybir.dt.float32
NP_BF16 = ml_dtypes.bfloat16

N = 1024          # tokens
C = 768           # channels
H = 12            # heads
DH = 64           # head dim
CT = C // 128     # 6 c-chunks
TT = N // 128     # 8 token tiles / key tiles
PAIRS = H // 2    # 6 head pairs


def _build_nc():
    nc = bacc.Bacc("TRN2", target_bir_lowering=False, debug=False)

    xt_d = nc.dram_tensor("xt", [C, N], BF16, kind="ExternalInput")
    wqkv_d = nc.dram_tensor("wqkv", [C, 3 * C], BF16, kind="ExternalInput")
    wproj_d = nc.dram_tensor("wproj", [C, C], BF16, kind="ExternalInput")
    bias_d = nc.dram_tensor("bias", [128, C], F32, kind="ExternalInput")
    y_d = nc.dram_tensor("y", [N, C], F32, kind="ExternalOutput")

    with tile.TileContext(nc) as tc:
        with tc.tile_pool(name="const", bufs=1) as cpool, \
             tc.tile_pool(name="qkv", bufs=1) as qkvpool, \
             tc.tile_pool(name="pt", bufs=32) as ptpool, \
             tc.tile_pool(name="norm", bufs=2) as npool, \
             tc.tile_pool(name="s", bufs=2, space="PSUM") as spool, \
             tc.tile_pool(name="acc", bufs=2, space="PSUM") as accpool, \
             tc.tile_pool(name="ins", bufs=2, space="PSUM") as ipool:

            # prepay the ACT exp table load
            dummy = cpool.tile([1, 2], F32, tag="dummy")
            nc.vector.memset(dummy[0:1, 0:1], 0.0)
            nc.scalar.activation(dummy[0:1, 1:2], dummy[0:1, 0:1],
                                 mybir.ActivationFunctionType.Exp)

            # ---- input DMA: few big dispatches, first-use order ----------
            # sync:   xt chunks 0,2,4 then wproj, bias (tail-only)
            # scalar: xt chunks 1,3,5 (ACT idle until ~15us)
            # gpsimd: W_qkv cols ft0 (q0), ft6 (k0), V cols, q-rest, k-rest
            xt_sb = cpool.tile([128, CT * N], BF16, tag="xt")
            x3 = xt_sb[:].rearrange("p (c n) -> p c n", n=N)
            for cc in range(CT):
                eng = nc.sync if cc % 2 == 0 else nc.scalar
                eng.dma_start(x3[:, cc, :], xt_d[cc * 128:(cc + 1) * 128, :])

            wq_sb = cpool.tile([128, CT * 3 * C], BF16, tag="wqkv")
            w3 = wq_sb[:].rearrange("p (c f) -> p c f", f=3 * C)

            def wdma(eng, c0, c1):
                eng.dma_start(
                    w3[:, :, c0:c1],
                    wqkv_d[:, c0:c1].rearrange("(c p) f -> p c f", p=128))

            wdma(nc.gpsimd, 0, 128)          # q features of pair 0
            wdma(nc.gpsimd, 768, 896)        # k features of pair 0
            wdma(nc.gpsimd, 1536, 2304)      # V columns
            wdma(nc.gpsimd, 128, 768)        # q features pairs 1-5
            wdma(nc.gpsimd, 896, 1536)       # k features pairs 1-5

            wp_sb = cpool.tile([128, CT * C], BF16, tag="wproj")
            wp3 = wp_sb[:].rearrange("p (c f) -> p c f", f=C)
            nc.sync.dma_start(
                wp3[:, :, :], wproj_d[:, :].rearrange("(c p) f -> p c f", p=128))
            bias_sb = cpool.tile([128, C], F32, tag="bias")
            nc.sync.dma_start(bias_sb[:], bias_d[:, :])

            # ---- persistent tiles ----------------------------------------
            qkT = [None] * 12          # ft 0..5 = q pairs, 6..11 = k pairs
            v_sb = [None] * TT         # [128, 12*65] = per head [V | ones]
            pt_tab = {}                # (f, jt) -> (pT_a, pT_b)
            outT = [qkvpool.tile([128, N], BF16, tag=f"outT{i}",
                                 name=f"outT{i}") for i in range(PAIRS)]

            # ---- building blocks -----------------------------------------
            def emit_qk_warm(ft):
                ps = spool.tile([128, N], F32, tag="s", name=f"qkw{ft}")
                t = qkvpool.tile([128, N], BF16, tag=f"qk{ft}", name=f"qk{ft}")
                qkT[ft] = t
                for half in range(2):
                    sl = slice(half * 512, (half + 1) * 512)
                    for cc in range(CT):
                        nc.tensor.matmul(
                            ps[:, sl],
                            lhsT=w3[:, cc, ft * 128:(ft + 1) * 128],
                            rhs=x3[:, cc, sl],
                            start=(cc == 0), stop=(cc == CT - 1))
                    nc.vector.tensor_copy(out=t[:, sl], in_=ps[:, sl])

            def emit_qk_insert_half(ft, half):
                ps = ipool.tile([128, 512], F32, tag="i", name=f"qk{ft}h{half}")
                sl = slice(half * 512, (half + 1) * 512)
                for cc in range(CT):
                    nc.tensor.matmul(
                        ps[:, :],
                        lhsT=w3[:, cc, ft * 128:(ft + 1) * 128],
                        rhs=x3[:, cc, sl],
                        start=(cc == 0), stop=(cc == CT - 1))
                if half == 0:
                    qkT[ft] = qkvpool.tile([128, N], BF16, tag=f"qk{ft}",
                                           name=f"qk{ft}")
                nc.vector.tensor_copy(out=qkT[ft][:, sl], in_=ps[:, :])

            v_ps = {}

            def emit_v_part(tt, part):
                """V for token tile tt on the insert-bank pair, as two
                cc-group units; the 512/256-col matmuls are interleaved
                per cc so the 256-col LDWEIGHTS hide."""
                if part == 0:
                    ps1 = ipool.tile([128, 512], F32, tag="i", name=f"v{tt}a")
                    ps2 = ipool.tile([128, 512], F32, tag="i", name=f"v{tt}b")
                    v_ps[tt] = (ps1, ps2)
                ps1, ps2 = v_ps[tt]
                for cc in (range(3) if part == 0 else range(3, CT)):
                    nc.tensor.matmul(
                        ps1[:, 0:512],
                        lhsT=x3[:, cc, tt * 128:(tt + 1) * 128],
                        rhs=w3[:, cc, 2 * C:2 * C + 512],
                        start=(cc == 0), stop=(cc == CT - 1))
                    nc.tensor.matmul(
                        ps2[:, 0:256],
                        lhsT=x3[:, cc, tt * 128:(tt + 1) * 128],
                        rhs=w3[:, cc, 2 * C + 512:3 * C],
                        start=(cc == 0), stop=(cc == CT - 1))
                if part == 1:
                    t = qkvpool.tile([128, H * 65], BF16, tag=f"v{tt}",
                                     name=f"v{tt}")
                    v_sb[tt] = t
                    t3 = t[:].rearrange("p (h w) -> p h w", w=65)
                    nc.vector.memset(t3[:, :, 64:65], 1.0)
                    nc.vector.tensor_copy(
                        out=t3[:, 0:8, 0:64],
                        in_=ps1[:, 0:512].rearrange("p (h w) -> p h w", w=64))
                    nc.vector.tensor_copy(
                        out=t3[:, 8:12, 0:64],
                        in_=ps2[:, 0:256].rearrange("p (h w) -> p h w", w=64))

            def emit_s_pair(f, jt):
                qt, kt = qkT[f], qkT[6 + f]
                a, b = 2 * f, 2 * f + 1
                ps_a = spool.tile([128, N], F32, tag="s", name=f"sa{f}_{jt}")
                ps_b = spool.tile([128, N], F32, tag="s", name=f"sb{f}_{jt}")
                jcols = slice(jt * 128, (jt + 1) * 128)
                for half in range(2):
                    sl = slice(half * 512, (half + 1) * 512)
                    nc.tensor.matmul(
                        ps_a[:, sl], lhsT=kt[0:64, jcols], rhs=qt[0:64, sl],
                        start=True, stop=True, tile_position=(0, 0))
                for half in range(2):
                    sl = slice(half * 512, (half + 1) * 512)
                    nc.tensor.matmul(
                        ps_b[:, sl], lhsT=kt[64:128, jcols], rhs=qt[64:128, sl],
                        start=True, stop=True, tile_position=(64, 0))
                pa = ptpool.tile([128, N], BF16, tag="pT", name=f"pT{a}_{jt}")
                nc.scalar.activation(pa[:], ps_a[:],
                                     mybir.ActivationFunctionType.Exp)
                pb = ptpool.tile([128, N], BF16, tag="pT", name=f"pT{b}_{jt}")
                nc.scalar.activation(pb[:], ps_b[:],
                                     mybir.ActivationFunctionType.Exp)
                pt_tab[(f, jt)] = (pa, pb)

            def emit_pv(p, jt, half, accs):
                """PV for both heads of pair p, key-tile jt, query half.
                acc[0:64] = out^T, acc[64] = denominator (ones col)."""
                hs = slice(half * 512, (half + 1) * 512)
                for hh in range(2):
                    h = 2 * p + hh
                    nc.tensor.matmul(
                        accs[hh][0:65, :],
                        lhsT=v_sb[jt][:, h * 65:(h + 1) * 65],
                        rhs=pt_tab[(p, jt)][hh][:, hs],
                        start=(jt == 0), stop=(jt == TT - 1))

            def norm_half(p, half, accs):
                den = npool.tile([1, N], F32, tag="den", name=f"den{p}_{half}")
                nc.vector.tensor_copy(out=den[0:1, 0:512], in_=accs[0][64:65, :])
                nc.vector.tensor_copy(out=den[0:1, 512:N], in_=accs[1][64:65, :])
                recip = npool.tile([1, N], F32, tag="recip",
                                   name=f"recip{p}_{half}")
                nc.vector.reciprocal_approx_fast(out=recip[:], in_=den[:])
                bc = npool.tile([64, N], F32, tag="bc", name=f"bc{p}_{half}")
                nc.gpsimd.partition_broadcast(bc[0:64, 0:512], recip[0:1, 0:512])
                nc.gpsimd.partition_broadcast(bc[0:64, 512:N], recip[0:1, 512:N])
                hc = slice(half * 512, (half + 1) * 512)
                nc.vector.tensor_tensor(
                    out=outT[p][0:64, hc], in0=accs[0][0:64, :],
                    in1=bc[0:64, 0:512], op=mybir.AluOpType.mult)
                nc.vector.tensor_tensor(
                    out=outT[p][64:128, hc], in0=accs[1][0:64, :],
                    in1=bc[0:64, 512:N], op=mybir.AluOpType.mult)

            # ---- filler deque --------------------------------------------
            fill = deque()   # items: (cost_cycles, closure, ready_pred|None)

            def drain(budget):
                while fill and budget > 0:
                    c, fn, ready = fill[0]
                    if ready is not None and not ready():
                        break
                    fill.popleft()
                    fn()
                    budget -= c
                return budget

            def ensure_qk(f):
                """Force-drain until pair f's q/k feature tiles exist
                (their insert units are always ahead in the deque)."""
                while qkT[f] is None or qkT[6 + f] is None:
                    c, fn, ready = fill.popleft()
                    fn()

            def push_pv_half(p, half):
                accs = [None, None]

                def mk_pv(jt):
                    def fn():
                        if jt == 0:
                            accs[0] = accpool.tile(
                                [128, 512], F32, tag="acc",
                                name=f"acc{2 * p}_{half}")
                            accs[1] = accpool.tile(
                                [128, 512], F32, tag="acc",
                                name=f"acc{2 * p + 1}_{half}")
                        emit_pv(p, jt, half, accs)
                    return fn
                for jt in range(TT):
                    fill.append((1024, mk_pv(jt),
                                 (lambda jt=jt: (p, jt) in pt_tab)))
                fill.append((0, lambda: norm_half(p, half, accs), None))

            def push_pv_phase(p):
                push_pv_half(p, 0)
                push_pv_half(p, 1)

            def push_inserts(f1):
                for ft in (f1, 6 + f1):
                    for half in range(2):
                        fill.append((3072, lambda ft=ft, half=half:
                                     emit_qk_insert_half(ft, half), None))

            def push_v(tt):
                fill.append((2304, lambda: emit_v_part(tt, 0), None))
                fill.append((2304, lambda: emit_v_part(tt, 1), None))

            # ---- projection helpers --------------------------------------
            proj_ps = {}

            def y_out(it, pieces):
                y_sb = npool.tile([128, C], F32, tag="ysb", bufs=3,
                                  name=f"y{it}")
                if len(pieces) == 1:
                    nc.vector.tensor_tensor(out=y_sb[:], in0=pieces[0],
                                            in1=bias_sb[:],
                                            op=mybir.AluOpType.add)
                else:
                    nc.vector.tensor_tensor(out=y_sb[:, 0:512], in0=pieces[0],
                                            in1=bias_sb[:, 0:512],
                                            op=mybir.AluOpType.add)
                    nc.vector.tensor_tensor(out=y_sb[:, 512:C], in0=pieces[1],
                                            in1=bias_sb[:, 512:C],
                                            op=mybir.AluOpType.add)
                dq = (nc.sync, nc.scalar, nc.gpsimd)[it % 3]
                dq.dma_start(y_d[it * 128:(it + 1) * 128, :], y_sb[:])

            def proj_start(it, ccs):
                ps = spool.tile([128, N], F32, tag="s", name=f"proj{it}")
                proj_ps[it] = ps
                for sl in (slice(0, 512), slice(512, 768)):
                    for cc in ccs:
                        nc.tensor.matmul(
                            ps[:, sl],
                            lhsT=outT[cc][:, it * 128:(it + 1) * 128],
                            rhs=wp3[:, cc, sl],
                            start=(cc == 0), stop=False)

            def proj_finish(it):
                ps = proj_ps[it]
                for sl in (slice(0, 512), slice(512, 768)):
                    nc.tensor.matmul(
                        ps[:, sl],
                        lhsT=outT[CT - 1][:, it * 128:(it + 1) * 128],
                        rhs=wp3[:, CT - 1, sl],
                        start=False, stop=True)
                y_out(it, (ps[:, 0:C],))

            def proj_start_pair(it, ps1, ps2):
                proj_ps[it] = (ps1, ps2)
                tcols = slice(it * 128, (it + 1) * 128)
                for ps, sl in ((ps1, slice(0, 512)), (ps2, slice(512, 768))):
                    w = sl.stop - sl.start
                    for cc in range(CT - 1):
                        nc.tensor.matmul(
                            ps[:, 0:w],
                            lhsT=outT[cc][:, tcols],
                            rhs=wp3[:, cc, sl],
                            start=(cc == 0), stop=False)

            def proj_finish_pair(it):
                ps1, ps2 = proj_ps[it]
                tcols = slice(it * 128, (it + 1) * 128)
                for ps, sl in ((ps1, slice(0, 512)), (ps2, slice(512, 768))):
                    w = sl.stop - sl.start
                    nc.tensor.matmul(
                        ps[:, 0:w],
                        lhsT=outT[CT - 1][:, tcols],
                        rhs=wp3[:, CT - 1, sl],
                        start=False, stop=True)
                y_out(it, (ps1[:, 0:512], ps2[:, 0:256]))

            def proj0_start():
                ps1 = ipool.tile([128, 512], F32, tag="i", name="proj0a")
                ps2 = ipool.tile([128, 512], F32, tag="i", name="proj0b")
                proj_start_pair(0, ps1, ps2)

            # ---- schedule ------------------------------------------------
            emit_qk_warm(0)
            emit_qk_warm(6)

            # exp-slot cadence: ACT does 2x1104ns per slot = ~5300 PE
            # cycles; S-pair itself is 2048 -> ~3300 filler cycles.  The
            # shortfall (deque dry or blocked) becomes HAM keep-alive
            # dummies so the PE activity window never cools.
            SLOT_FILLER = 3300

            for f in range(PAIRS):
                if f == 0:
                    push_v(0)
                    push_inserts(1)
                    for tt in range(1, 5):
                        push_v(tt)
                elif f == 1:
                    for tt in range(5, TT):
                        push_v(tt)
                if 1 <= f < PAIRS - 1:
                    push_inserts(f + 1)
                if f >= 1:
                    push_pv_phase(f - 1)
                if f == PAIRS - 1:
                    fill.append((3840, proj0_start, None))
                ensure_qk(f)
                for jt in range(TT):
                    if f == 0 and jt <= 1:
                        emit_s_pair(f, jt)
                        drain(SLOT_FILLER - 2048)
                    else:
                        drain(SLOT_FILLER)
                        emit_s_pair(f, jt)
                    if f == PAIRS - 1 and jt == 1:
                        push_pv_half(PAIRS - 1, 0)

            # last pair's remaining PV + the projection tail
            push_pv_half(PAIRS - 1, 1)
            drain(1 << 30)
            proj_start(1, range(CT - 1))
            proj_start(2, range(CT - 1))
            proj_finish_pair(0)
            acc3a = accpool.tile([128, 512], F32, tag="acc", name="proj3a")
            acc3b = accpool.tile([128, 512], F32, tag="acc", name="proj3b")
            proj_start_pair(3, acc3a, acc3b)
            ps4a = ipool.tile([128, 512], F32, tag="i", name="proj4a")
            ps4b = ipool.tile([128, 512], F32, tag="i", name="proj4b")
            proj_start_pair(4, ps4a, ps4b)
            proj_finish(1)
            proj_start(5, range(CT - 1))
            proj_finish(2)
            proj_start(6, range(CT - 1))
            proj_finish_pair(3)
            acc7a = accpool.tile([128, 512], F32, tag="acc", name="proj7a")
            acc7b = accpool.tile([128, 512], F32, tag="acc", name="proj7b")
            proj_start_pair(7, acc7a, acc7b)
            proj_finish_pair(4)
            proj_finish(5)
            proj_finish(6)
            proj_finish_pair(7)

    nc.compile()
    return nc


_NC_CACHE = None


def _get_nc():
    global _NC_CACHE
    if _NC_CACHE is None:
        _NC_CACHE = _build_nc()
    return _NC_CACHE


def kernel(x, geometric, text, color, W_qkv, W_proj, b_proj,
           geo_bias, txt_bias, col_bias, _trace=False, **_ignored):
    x = np.asarray(x, dtype=np.float32)
    W_qkv = np.asarray(W_qkv, dtype=np.float32)
    W_proj = np.asarray(W_proj, dtype=np.float32)
    b_proj = np.asarray(b_proj, dtype=np.float32)

    scale = DH ** -0.5
    wqkv = W_qkv.copy()
    wqkv[:, :C] *= scale
    wqkv_bf = wqkv.astype(NP_BF16)
    wproj_bf = W_proj.astype(NP_BF16)
    bias_f = np.ascontiguousarray(
        np.broadcast_to(b_proj, (128, C))).astype(np.float32)

    in_maps = []
    for b in range(8):
        xt = np.ascontiguousarray(x[b].T).astype(NP_BF16)
        in_maps.append({"xt": xt, "wqkv": wqkv_bf, "wproj": wproj_bf,
                        "bias": bias_f})

    nc = _get_nc()
    res = run_bass_kernel_spmd(nc, in_maps, core_ids=list(range(8)),
                               trace=_trace)
    y = np.stack([r["y"] for r in res.results]).astype(np.float32)
    if _trace:
        kernel.last_results = res
    return y
